# revision 1
# baseline (speedup 1.0000x reference)
"""Multi-head attention Trainium2 Bass kernel (8 NeuronCores).

Problem: B=2, S=2048, D=1024, H=16 heads, dh=64.
  q = (X_q @ Wq), k = (X_k @ Wk), v = (X_v @ Wv)   (per-head split)
  out = softmax(q k^T / sqrt(dh)) v, concat heads, @ Wo

Sharding: 8 cores = 2 batches x 4 head-groups (4 heads each).
Core c handles batch c//4, heads [4*(c%4), 4*(c%4)+4).
Each core computes a partial output y_c = attn_out_c @ Wo[rows_c]; the host
sums the 4 partials per batch (tensor-parallel unshard).

Per-core layouts (host pre-transposes X so the contraction dim D lands on
SBUF partitions; no on-device transposes anywhere):
  xq/xk/xv : [8, 128, 2048]  = X^T chunked by D        (f32r)
  wq/wk/wv : [8, 128, 256]   = W[:, group-cols] by D   (f32r)
  wo       : [2, 128, 1024]  = Wo[group-rows, :]       (f32r)
  y        : [16, 128, 1024] = partial output by S     (f32)

Algorithm per head (no transposes anywhere):
  scoresT[k, q] via lhsT=kT slice, rhs=qT slice (K=dh=64)
  P^T = exp(0.125 * scoresT)  (ACT, PSUM->SBUF, f32r; 16-bit moving
  operands would force a Ldweights split per matmul).  Softmax without
  max-subtraction: scores ~ N(0,1), exp never overflows.
  U_aug[65, q] = sum_k v_aug[k, 65]^T P^T[k, q]; v_aug has a ones column
  so row 64 = softmax denominators l.
  U = U_aug[0:64] * bcast(1/l);  y = U(as lhsT) @ Wo with K=256 fused.

Backend notes (axon emulator, not real silicon): each instruction costs
~25-45us fixed plus a data-size term, so the design minimizes instruction
count: 4-bank [128, 2048] PSUM tiles filled by 512-wide bank-aligned
matmul slices (matmul start=True zeroes the WHOLE 2KB bank, so slices
narrower than a bank corrupt neighbours), one wide Exp per key-chunk,
flat partition-major DMA layouts (host pre-transposes), batched output
DMAs.  ~1146 instructions/rep vs 1635 for the naive tiling.
"""
import sys

sys.path.insert(0, "/opt/trn_rl_repo")

import numpy as np

B, S, D, H, DH = 2, 2048, 1024, 16, 64
NCORES = 8
GROUPS = 4          # head-groups (tensor-parallel dim)
HPG = H // GROUPS   # heads per group = 4
GC = HPG * DH       # group cols = 256
KC_D = D // 128     # 8  D-chunks
KC_S = S // 128     # 16 S-chunks

_CACHE = {}


def build_program(reps=1, phases="123", debug_dumps=False):
    from concourse import bacc, tile, mybir
    from concourse.masks import make_identity

    DT = mybir.dt.float32r
    BF = mybir.dt.bfloat16
    F32 = mybir.dt.float32
    EXP = mybir.ActivationFunctionType.Exp

    nc = bacc.Bacc("TRN2", target_bir_lowering=False, debug=False,
                   num_devices=NCORES)
    xq = nc.dram_tensor("xq", [128, KC_D, S], DT, kind="ExternalInput").ap()
    xk = nc.dram_tensor("xk", [128, KC_D, S], DT, kind="ExternalInput").ap()
    xv = nc.dram_tensor("xv", [128, KC_D, S], DT, kind="ExternalInput").ap()
    wq = nc.dram_tensor("wq", [128, KC_D, GC], DT, kind="ExternalInput").ap()
    wk = nc.dram_tensor("wk", [128, KC_D, GC], DT, kind="ExternalInput").ap()
    wv = nc.dram_tensor("wv", [128, KC_D, GC], DT, kind="ExternalInput").ap()
    wo = nc.dram_tensor("wo", [128, 2, D], DT, kind="ExternalInput").ap()
    y = nc.dram_tensor("y", [128, KC_S, D], F32, kind="ExternalOutput").ap()
    if debug_dumps:
        dq = nc.dram_tensor("dq", [2, 128, S], DT, kind="ExternalOutput").ap()
        dk = nc.dram_tensor("dk", [2, 128, S], DT, kind="ExternalOutput").ap()
        du = nc.dram_tensor("du", [2, 128, S], DT, kind="ExternalOutput").ap()
        dv = nc.dram_tensor("dv", [128, KC_S * HPG * (DH + 1)], DT,
                            kind="ExternalOutput").ap()

    with tile.TileContext(nc) as tc:
        with (
            tc.tile_pool(name="persist", bufs=1) as persist,
        ):
            # weights resident across phases
            w_sb = {}
            for nm, wd in (("wq", wq), ("wk", wk), ("wv", wv)):
                w_sb[nm] = persist.tile([128, KC_D, GC], DT, tag=nm, name=nm)
            wo_sb = persist.tile([128, 2, D], DT, tag="wo")
            ones_c = persist.tile([128, KC_S * HPG], F32, tag="ones")
            nc.any.memset(ones_c[:], 1.0)
            idf = persist.tile([128, 128], F32, tag="idf")
            make_identity(nc, idf[:])
            ident = persist.tile([128, 128], DT, tag="ident")
            nc.vector.tensor_copy(ident[:], idf[:])

            for _ in range(reps):
                qt = [persist.tile([128, S], DT, tag=f"qt{i}", name=f"qt{i}")
                      for i in range(2)]
                kt = [persist.tile([128, S], DT, tag=f"kt{i}", name=f"kt{i}")
                      for i in range(2)]
                ut = [persist.tile([128, S], DT, tag=f"ut{i}", name=f"ut{i}")
                      for i in range(2)]
                # v_aug: [keys-in-chunk, chunk, head, 64 v-dims + ones]
                v_sb = persist.tile([128, KC_S, HPG, DH + 1], DT, tag="v")

                if "1" in phases:
                    with (
                        tc.tile_pool(name="xfull", bufs=1) as xf_pool,
                        tc.tile_pool(name="psum_p", bufs=2,
                                     space="PSUM") as psum_p,
                    ):
                        nc.sync.dma_start(out=w_sb["wq"][:], in_=wq)
                        nc.sync.dma_start(out=w_sb["wk"][:], in_=wk)
                        nc.sync.dma_start(out=w_sb["wv"][:], in_=wv)

                        # ---- Q and K projections: psum [128, 2048] per
                        # M-tile, 4 bank-aligned N-slices x 8 K-chunks ----
                        for nm, xd, dst in (("wq", xq, qt), ("wk", xk, kt)):
                            xst = xf_pool.tile([128, KC_D, S], DT, tag="xf",
                                               name="xf")
                            nc.sync.dma_start(out=xst[:], in_=xd)
                            for m in range(2):
                                ps = psum_p.tile([128, S], F32, tag="pp",
                                                 name="pp")
                                for j in range(4):
                                    for kc in range(KC_D):
                                        nc.tensor.matmul(
                                            ps[:, j * 512:(j + 1) * 512],
                                            w_sb[nm][:, kc,
                                                     m * 128:(m + 1) * 128],
                                            xst[:, kc, j * 512:(j + 1) * 512],
                                            start=(kc == 0),
                                            stop=(kc == KC_D - 1),
                                            skip_group_check=True)
                                nc.vector.tensor_copy(dst[m][:], ps[:])
                        # ---- V projection, transposed form like Q/K
                        # (vt[m] rows = group cols, cols = keys), then PE
                        # transposes to v_sb[keys, ...].  Transpose slots are
                        # 512B, 4 per PSUM bank: start=True (bank zero) only
                        # on the first slot of each bank. ----
                        xst = xf_pool.tile([128, KC_D, S], DT, tag="xf",
                                           name="xf")
                        nc.sync.dma_start(out=xst[:], in_=xv)
                        vt = [xf_pool.tile([128, S], DT, tag=f"vt{m}",
                                           name="vt") for m in range(2)]
                        for m in range(2):
                            ps = psum_p.tile([128, S], F32, tag="pp",
                                             name="pv")
                            for j in range(4):
                                for kc in range(KC_D):
                                    nc.tensor.matmul(
                                        ps[:, j * 512:(j + 1) * 512],
                                        w_sb["wv"][:, kc,
                                                   m * 128:(m + 1) * 128],
                                        xst[:, kc, j * 512:(j + 1) * 512],
                                        start=(kc == 0),
                                        stop=(kc == KC_D - 1),
                                        skip_group_check=True)
                            nc.vector.tensor_copy(vt[m][:], ps[:])
                        for t in range(2):
                            pst = psum_p.tile([128, S], DT, tag="pp",
                                              name="ptr")
                            for c8 in range(8):
                                c = t * 8 + c8
                                for m in range(2):
                                    p = c8 * 2 + m
                                    nc.tensor.matmul(
                                        pst[:, p * 128:(p + 1) * 128],
                                        vt[m][:, c * 128:(c + 1) * 128],
                                        ident[:],
                                        is_transpose=True,
                                        start=(p % 4 == 0),
                                        stop=(p % 4 == 3),
                                        skip_group_check=True)
                            nc.vector.tensor_copy(
                                v_sb[:, t * 8:(t + 1) * 8, :, 0:DH],
                                pst.rearrange("p (c h d) -> p c h d",
                                              c=8, h=HPG))
                        nc.vector.tensor_copy(
                            v_sb[:, :, :, DH:DH + 1],
                            ones_c.rearrange("p (c h one) -> p c h one",
                                             c=KC_S, one=1))
                if "2" in phases:
                    with (
                        tc.tile_pool(name="pt", bufs=8) as pt_pool,
                        tc.tile_pool(name="norm", bufs=1) as norm_pool,
                        tc.tile_pool(name="psum_sc", bufs=1,
                                     space="PSUM") as psum_sc,
                        tc.tile_pool(name="psum_u", bufs=1,
                                     space="PSUM") as psum_u,
                    ):
                        for h in range(HPG):
                            ktile, row = h // 2, (h % 2) * 64
                            if h == HPG - 1 and "3" in phases:
                                nc.sync.dma_start(out=wo_sb[:], in_=wo)
                            up = psum_u.tile([65, S], F32, tag="u", name="u")
                            for half in range(2):
                                pts = []
                                for ci in range(8):
                                    c = half * 8 + ci
                                    ps = psum_sc.tile([128, S], F32, tag="sc",
                                                      name="sc")
                                    for j in range(4):
                                        nc.tensor.matmul(
                                            ps[:, j * 512:(j + 1) * 512],
                                            kt[ktile][row:row + 64,
                                                      c * 128:(c + 1) * 128],
                                            qt[ktile][row:row + 64,
                                                      j * 512:(j + 1) * 512],
                                            start=True, stop=True)
                                    pt = pt_pool.tile([128, S], DT, tag="pt",
                                                      name="pt")
                                    nc.scalar.activation(pt[:], ps[:],
                                                         EXP, scale=0.125)
                                    pts.append(pt)
                                for j in range(4):
                                    for ci in range(8):
                                        c = half * 8 + ci
                                        nc.tensor.matmul(
                                            up[:, j * 512:(j + 1) * 512],
                                            v_sb[:, c, h, :],
                                            pts[ci][:, j * 512:(j + 1) * 512],
                                            start=(c == 0), stop=(c == KC_S - 1),
                                            skip_group_check=True)
                            rl = norm_pool.tile([1, S], F32, tag="rl",
                                                name="rl")
                            rlb = norm_pool.tile([64, S], F32, tag="rlb",
                                                 name="rlb")
                            nc.vector.reciprocal(rl[:], up[64:65, :])
                            nc.gpsimd.partition_broadcast(rlb[:], rl[:])
                            nc.vector.tensor_mul(
                                ut[ktile][row:row + 64, :], up[0:64, :],
                                rlb[:])

                if debug_dumps:
                    for i in range(2):
                        nc.sync.dma_start(out=dq[i], in_=qt[i][:])
                        nc.sync.dma_start(out=dk[i], in_=kt[i][:])
                        nc.sync.dma_start(out=du[i], in_=ut[i][:])
                    nc.sync.dma_start(
                        out=dv[:], in_=v_sb.rearrange("p a b c -> p (a b c)"))

                # ---- output projection y = U(lhsT) @ Wo ----
                if "3" in phases:
                    with (
                        tc.tile_pool(name="yst", bufs=2) as y_pool,
                        tc.tile_pool(name="psum_y", bufs=2,
                                     space="PSUM") as psum_y,
                    ):
                        if "2" not in phases:
                            nc.sync.dma_start(out=wo_sb[:], in_=wo)
                        for g in range(4):
                            ys = y_pool.tile([128, 4, D], F32, tag="ys",
                                             name="ys")
                            for pr in range(2):
                                ps = psum_y.tile([128, 2, D], F32, tag="py",
                                                 name="py")
                                for si in range(2):
                                    sc = g * 4 + pr * 2 + si
                                    for dc in range(2):
                                        for ktile in range(2):
                                            nc.tensor.matmul(
                                                ps[:, si,
                                                   dc * 512:(dc + 1) * 512],
                                                ut[ktile][:, sc * 128:
                                                          (sc + 1) * 128],
                                                wo_sb[:, ktile,
                                                      dc * 512:(dc + 1) * 512],
                                                start=(ktile == 0),
                                                stop=(ktile == 1),
                                                skip_group_check=True)
                                nc.vector.tensor_copy(
                                    ys[:, pr * 2:(pr + 1) * 2, :], ps[:])
                            nc.sync.dma_start(
                                out=y[:, g * 4:(g + 1) * 4, :], in_=ys[:])

    nc.compile()
    return nc


def _pm(a, kc):
    """[ (kc*128), m ] -> partition-major [128, kc, m]."""
    m = a.shape[-1]
    return np.ascontiguousarray(a.reshape(kc, 128, m).transpose(1, 0, 2))


def _prep_inputs(queries, keys, values, Wq, Wk, Wv, Wo):
    """Shard: per core (batch b, group g) -> input map."""
    qT = [_pm(queries[b].T, KC_D) for b in range(B)]
    kT = [_pm(keys[b].T, KC_D) for b in range(B)]
    vT = [_pm(values[b].T, KC_D) for b in range(B)]
    in_maps = []
    for c in range(NCORES):
        b, g = c // GROUPS, c % GROUPS
        cols = slice(g * GC, (g + 1) * GC)
        in_maps.append({
            "xq": qT[b],
            "xk": kT[b],
            "xv": vT[b],
            "wq": _pm(Wq[:, cols], KC_D),
            "wk": _pm(Wk[:, cols], KC_D),
            "wv": _pm(Wv[:, cols], KC_D),
            "wo": _pm(Wo[cols, :], 2),
        })
    return in_maps


def kernel(queries, keys, values, Wq, Wk, Wv, Wo):
    from concourse.bass_utils import run_bass_kernel_spmd

    queries = np.asarray(queries, dtype=np.float32)
    keys = np.asarray(keys, dtype=np.float32)
    values = np.asarray(values, dtype=np.float32)
    Wq = np.asarray(Wq, dtype=np.float32)
    Wk = np.asarray(Wk, dtype=np.float32)
    Wv = np.asarray(Wv, dtype=np.float32)
    Wo = np.asarray(Wo, dtype=np.float32)

    if "nc" not in _CACHE:
        _CACHE["nc"] = build_program()
    nc = _CACHE["nc"]

    in_maps = _prep_inputs(queries, keys, values, Wq, Wk, Wv, Wo)
    res = None
    for attempt in range(3):
        try:
            res = run_bass_kernel_spmd(nc, in_maps, list(range(NCORES)))
            break
        except Exception:
            if attempt == 2:
                raise
            import time
            time.sleep(2.0)

    out = np.zeros((B, S, D), dtype=np.float32)
    for c in range(NCORES):
        b = c // GROUPS
        out[b] += res.results[c]["y"].reshape(128, KC_S, D).transpose(
            1, 0, 2).reshape(S, D)
    return out



# revision 19
# speedup vs baseline: 1.0517x; 1.0517x over previous
"""Multi-head attention Trainium2 Bass kernel (8 NeuronCores).

Problem: B=2, S=2048, D=1024, H=16 heads, dh=64.
  q = (X_q @ Wq), k = (X_k @ Wk), v = (X_v @ Wv)   (per-head split)
  out = softmax(q k^T / sqrt(dh)) v, concat heads, @ Wo

Sharding: 8 cores = 2 batches x 4 head-groups (4 heads each).
Core c handles batch c//4, heads [4*(c%4), 4*(c%4)+4).
Each core computes a partial output y_c = attn_out_c @ Wo[rows_c]; the host
sums the 4 partials per batch (tensor-parallel unshard).

Per-core layouts (host pre-transposes X so the contraction dim D lands on
SBUF partitions; no on-device transposes anywhere):
  xq/xk/xv : [8, 128, 2048]  = X^T chunked by D        (f32r)
  wq/wk/wv : [8, 128, 256]   = W[:, group-cols] by D   (f32r)
  wo       : [2, 128, 1024]  = Wo[group-rows, :]       (f32r)
  y        : [16, 128, 1024] = partial output by S     (f32)

Algorithm per head (no transposes anywhere):
  scoresT[k, q] via lhsT=kT slice, rhs=qT slice (K=dh=64)
  P^T = exp(0.125 * scoresT)  (ACT, PSUM->SBUF, f32r; 16-bit moving
  operands would force a Ldweights split per matmul).  Softmax without
  max-subtraction: scores ~ N(0,1), exp never overflows.
  U_aug[65, q] = sum_k v_aug[k, 65]^T P^T[k, q]; v_aug has a ones column
  so row 64 = softmax denominators l.
  U = U_aug[0:64] * bcast(1/l);  y = U(as lhsT) @ Wo with K=256 fused.

Backend notes (axon emulator, not real silicon): each instruction costs
~25-45us fixed plus a data-size term, so the design minimizes instruction
count: 4-bank [128, 2048] PSUM tiles filled by 512-wide bank-aligned
matmul slices (matmul start=True zeroes the WHOLE 2KB bank, so slices
narrower than a bank corrupt neighbours), one wide Exp per key-chunk,
flat partition-major DMA layouts (host pre-transposes), batched output
DMAs.  ~1146 instructions/rep vs 1635 for the naive tiling.
"""
import sys

sys.path.insert(0, "/opt/trn_rl_repo")

import numpy as np

B, S, D, H, DH = 2, 2048, 1024, 16, 64
NCORES = 8
GROUPS = 4          # head-groups (tensor-parallel dim)
HPG = H // GROUPS   # heads per group = 4
GC = HPG * DH       # group cols = 256
KC_D = D // 128     # 8  D-chunks
KC_S = S // 128     # 16 S-chunks

_CACHE = {}


def build_program(reps=1, phases="123", debug_dumps=False):
    from concourse import bacc, tile, mybir
    from concourse.masks import make_identity

    DT = mybir.dt.float32r
    BF = mybir.dt.bfloat16
    F32 = mybir.dt.float32
    EXP = mybir.ActivationFunctionType.Exp
    CPY = mybir.ActivationFunctionType.Copy

    nc = bacc.Bacc("TRN2", target_bir_lowering=False, debug=False,
                   num_devices=NCORES)
    xq = nc.dram_tensor("xq", [128, KC_D, S], DT, kind="ExternalInput").ap()
    xk = nc.dram_tensor("xk", [128, KC_D, S], DT, kind="ExternalInput").ap()
    xv = nc.dram_tensor("xv", [128, KC_D, S], DT, kind="ExternalInput").ap()
    # packed qkv weights: [:, kc, nm*GC:(nm+1)*GC] = W{q,k,v}[:, cols] chunk
    wqkv = nc.dram_tensor("wqkv", [128, KC_D, 3 * GC], DT,
                          kind="ExternalInput").ap()
    wo = nc.dram_tensor("wo", [128, 2, D], DT, kind="ExternalInput").ap()
    y = nc.dram_tensor("y", [128, KC_S, D], F32, kind="ExternalOutput").ap()
    if debug_dumps:
        dq = nc.dram_tensor("dq", [2, 128, S], DT, kind="ExternalOutput").ap()
        dk = nc.dram_tensor("dk", [2, 128, S], DT, kind="ExternalOutput").ap()
        du = nc.dram_tensor("du", [2, 128, S], DT, kind="ExternalOutput").ap()
        dv = nc.dram_tensor("dv", [128, KC_S * HPG * (DH + 1)], DT,
                            kind="ExternalOutput").ap()

    with tile.TileContext(nc) as tc:
        with (
            tc.tile_pool(name="persist", bufs=1) as persist,
        ):
            # weights resident across phases (one packed tile, one DMA)
            wqkv_sb = persist.tile([128, KC_D, 3 * GC], DT, tag="wqkv",
                                   name="wqkv")
            w_sb = {nm: wqkv_sb[:, :, i * GC:(i + 1) * GC]
                    for i, nm in enumerate(("wq", "wk", "wv"))}
            wo_sb = persist.tile([128, 2, D], DT, tag="wo")
            ones_c = persist.tile([128, KC_S * HPG], F32, tag="ones")
            nc.any.memset(ones_c[:], 1.0)
            idf = persist.tile([128, 128], F32, tag="idf")
            make_identity(nc, idf[:])
            ident = persist.tile([128, 128], DT, tag="ident")
            nc.vector.tensor_copy(ident[:], idf[:])

            for _ in range(reps):
                qt = [persist.tile([128, S], DT, tag=f"qt{i}", name=f"qt{i}")
                      for i in range(2)]
                kt = [persist.tile([128, S], DT, tag=f"kt{i}", name=f"kt{i}")
                      for i in range(2)]
                ut = [persist.tile([128, S], DT, tag=f"ut{i}", name=f"ut{i}")
                      for i in range(2)]
                # v_aug: [keys-in-chunk, chunk, head, 64 v-dims + ones]
                v_sb = persist.tile([128, KC_S, HPG, DH + 1], DT, tag="v")

                if "1" in phases:
                    with (
                        tc.tile_pool(name="xfull", bufs=1) as xf_pool,
                        tc.tile_pool(name="psum_p", bufs=2,
                                     space="PSUM") as psum_p,
                    ):
                        nc.sync.dma_start(out=wqkv_sb[:], in_=wqkv)

                        # ---- Q and K projections: psum [128, 2048] per
                        # M-tile, 4 bank-aligned N-slices x 8 K-chunks ----
                        for nm, xd, dst in (("wq", xq, qt), ("wk", xk, kt)):
                            xst = xf_pool.tile([128, KC_D, S], DT, tag="xf",
                                               name="xf")
                            nc.sync.dma_start(out=xst[:], in_=xd)
                            for m in range(2):
                                ps = psum_p.tile([128, S], F32, tag="pp",
                                                 name="pp")
                                for j in range(4):
                                    for kc in range(KC_D):
                                        nc.tensor.matmul(
                                            ps[:, j * 512:(j + 1) * 512],
                                            w_sb[nm][:, kc,
                                                     m * 128:(m + 1) * 128],
                                            xst[:, kc, j * 512:(j + 1) * 512],
                                            start=(kc == 0),
                                            stop=(kc == KC_D - 1),
                                            skip_group_check=True)
                                nc.scalar.activation(dst[m][:], ps[:],
                                                     CPY, scale=1.0)
                        # ---- V projection, transposed form like Q/K
                        # (vt[m] rows = group cols, cols = keys), then PE
                        # transposes to v_sb[keys, ...].  Transpose slots are
                        # 512B, 4 per PSUM bank: start=True (bank zero) only
                        # on the first slot of each bank. ----
                        xst = xf_pool.tile([128, KC_D, S], DT, tag="xf",
                                           name="xf")
                        nc.sync.dma_start(out=xst[:], in_=xv)
                        vt = [xf_pool.tile([128, S], DT, tag=f"vt{m}",
                                           name="vt") for m in range(2)]
                        for m in range(2):
                            ps = psum_p.tile([128, S], F32, tag="pp",
                                             name="pv")
                            for j in range(4):
                                for kc in range(KC_D):
                                    nc.tensor.matmul(
                                        ps[:, j * 512:(j + 1) * 512],
                                        w_sb["wv"][:, kc,
                                                   m * 128:(m + 1) * 128],
                                        xst[:, kc, j * 512:(j + 1) * 512],
                                        start=(kc == 0),
                                        stop=(kc == KC_D - 1),
                                        skip_group_check=True)
                            nc.scalar.activation(vt[m][:], ps[:],
                                                 CPY, scale=1.0)
                        for t in range(2):
                            pst = psum_p.tile([128, S], DT, tag="pp",
                                              name="ptr")
                            for c8 in range(8):
                                c = t * 8 + c8
                                for m in range(2):
                                    p = c8 * 2 + m
                                    nc.tensor.matmul(
                                        pst[:, p * 128:(p + 1) * 128],
                                        vt[m][:, c * 128:(c + 1) * 128],
                                        ident[:],
                                        is_transpose=True,
                                        start=(p % 4 == 0),
                                        stop=(p % 4 == 3),
                                        skip_group_check=True)
                            nc.scalar.activation(
                                v_sb[:, t * 8:(t + 1) * 8, :, 0:DH],
                                pst.rearrange("p (c h d) -> p c h d",
                                              c=8, h=HPG),
                                CPY, scale=1.0)
                        nc.vector.tensor_copy(
                            v_sb[:, :, :, DH:DH + 1],
                            ones_c.rearrange("p (c h one) -> p c h one",
                                             c=KC_S, one=1))
                if "2" in phases:
                    do_pv = "4" not in phases
                    with (
                        tc.tile_pool(name="pt", bufs=8) as pt_pool,
                        tc.tile_pool(name="norm", bufs=1) as norm_pool,
                        tc.tile_pool(name="psum_sc", bufs=1,
                                     space="PSUM") as psum_sc,
                        tc.tile_pool(name="psum_u", bufs=1,
                                     space="PSUM") as psum_u,
                    ):
                        for h in range(HPG):
                            ktile, row = h // 2, (h % 2) * 64
                            if h == HPG - 1 and "3" in phases:
                                nc.sync.dma_start(out=wo_sb[:], in_=wo)
                            if do_pv:
                                up = psum_u.tile([65, S], F32, tag="u",
                                                 name="u")
                            for half in range(2):
                                pts = []
                                for ci in range(8):
                                    c = half * 8 + ci
                                    ps = psum_sc.tile([128, S], F32, tag="sc",
                                                      name="sc")
                                    for j in range(4):
                                        nc.tensor.matmul(
                                            ps[:, j * 512:(j + 1) * 512],
                                            kt[ktile][row:row + 64,
                                                      c * 128:(c + 1) * 128],
                                            qt[ktile][row:row + 64,
                                                      j * 512:(j + 1) * 512],
                                            start=True, stop=True)
                                    pt = pt_pool.tile([128, S], DT, tag="pt",
                                                      name="pt")
                                    nc.scalar.activation(pt[:], ps[:],
                                                         EXP, scale=0.125)
                                    pts.append(pt)
                                if do_pv:
                                    for j in range(4):
                                        for ci in range(8):
                                            c = half * 8 + ci
                                            nc.tensor.matmul(
                                                up[:, j * 512:(j + 1) * 512],
                                                v_sb[:, c, h, :],
                                                pts[ci][:, j * 512:(j + 1) * 512],
                                                start=(c == 0),
                                                stop=(c == KC_S - 1),
                                                skip_group_check=True)
                            if do_pv:
                                rl = norm_pool.tile([1, S], F32, tag="rl",
                                                    name="rl")
                                rlb = norm_pool.tile([64, S], F32, tag="rlb",
                                                     name="rlb")
                                nc.vector.reciprocal(rl[:], up[64:65, :])
                                nc.gpsimd.partition_broadcast(rlb[:], rl[:])
                                nc.vector.tensor_mul(
                                    ut[ktile][row:row + 64, :], up[0:64, :],
                                    rlb[:])

                if debug_dumps:
                    for i in range(2):
                        nc.sync.dma_start(out=dq[i], in_=qt[i][:])
                        nc.sync.dma_start(out=dk[i], in_=kt[i][:])
                        nc.sync.dma_start(out=du[i], in_=ut[i][:])
                    nc.sync.dma_start(
                        out=dv[:], in_=v_sb.rearrange("p a b c -> p (a b c)"))

                # ---- output projection y = U(lhsT) @ Wo ----
                if "3" in phases:
                    with (
                        tc.tile_pool(name="yst", bufs=1) as y_pool,
                        tc.tile_pool(name="psum_y", bufs=2,
                                     space="PSUM") as psum_y,
                    ):
                        if "2" not in phases:
                            nc.sync.dma_start(out=wo_sb[:], in_=wo)
                        ys = y_pool.tile([128, KC_S, D], F32, tag="ys",
                                         name="ys")
                        for g in range(4):
                            for pr in range(2):
                                ps = psum_y.tile([128, 2, D], F32, tag="py",
                                                 name="py")
                                for si in range(2):
                                    sc = g * 4 + pr * 2 + si
                                    for dc in range(2):
                                        for ktile in range(2):
                                            nc.tensor.matmul(
                                                ps[:, si,
                                                   dc * 512:(dc + 1) * 512],
                                                ut[ktile][:, sc * 128:
                                                          (sc + 1) * 128],
                                                wo_sb[:, ktile,
                                                      dc * 512:(dc + 1) * 512],
                                                start=(ktile == 0),
                                                stop=(ktile == 1),
                                                skip_group_check=True)
                                nc.scalar.activation(
                                    ys[:, g * 4 + pr * 2:
                                       g * 4 + (pr + 1) * 2, :],
                                    ps[:], CPY, scale=1.0)
                        nc.sync.dma_start(out=y, in_=ys[:])

    nc.compile()
    return nc


def _pm(a, kc):
    """[ (kc*128), m ] -> partition-major [128, kc, m]."""
    m = a.shape[-1]
    return np.ascontiguousarray(a.reshape(kc, 128, m).transpose(1, 0, 2))


def _prep_inputs(queries, keys, values, Wq, Wk, Wv, Wo):
    """Shard: per core (batch b, group g) -> input map."""
    qT = [_pm(queries[b].T, KC_D) for b in range(B)]
    kT = [_pm(keys[b].T, KC_D) for b in range(B)]
    vT = [_pm(values[b].T, KC_D) for b in range(B)]
    in_maps = []
    for c in range(NCORES):
        b, g = c // GROUPS, c % GROUPS
        cols = slice(g * GC, (g + 1) * GC)
        in_maps.append({
            "xq": qT[b],
            "xk": kT[b],
            "xv": vT[b],
            "wqkv": np.ascontiguousarray(np.concatenate(
                [_pm(Wq[:, cols], KC_D), _pm(Wk[:, cols], KC_D),
                 _pm(Wv[:, cols], KC_D)], axis=2)),
            "wo": _pm(Wo[cols, :], 2),
        })
    return in_maps


def kernel(queries, keys, values, Wq, Wk, Wv, Wo):
    from concourse.bass_utils import run_bass_kernel_spmd

    queries = np.asarray(queries, dtype=np.float32)
    keys = np.asarray(keys, dtype=np.float32)
    values = np.asarray(values, dtype=np.float32)
    Wq = np.asarray(Wq, dtype=np.float32)
    Wk = np.asarray(Wk, dtype=np.float32)
    Wv = np.asarray(Wv, dtype=np.float32)
    Wo = np.asarray(Wo, dtype=np.float32)

    if "nc" not in _CACHE:
        _CACHE["nc"] = build_program()
    nc = _CACHE["nc"]

    in_maps = _prep_inputs(queries, keys, values, Wq, Wk, Wv, Wo)
    res = None
    for attempt in range(3):
        try:
            res = run_bass_kernel_spmd(nc, in_maps, list(range(NCORES)))
            break
        except Exception:
            if attempt == 2:
                raise
            import time
            time.sleep(2.0)

    out = np.zeros((B, S, D), dtype=np.float32)
    for c in range(NCORES):
        b = c // GROUPS
        out[b] += res.results[c]["y"].reshape(128, KC_S, D).transpose(
            1, 0, 2).reshape(S, D)
    return out

